# revision 28
# baseline (speedup 1.0000x reference)
"""Trainium2 Bass kernel for nn_DenseTensor (dense_mlp, bilinear form).

Computes out = x @ W + einsum('bd,due,be->bu', x, V, x) + b with
B=1024, D=U=E=512 on 8 NeuronCores.

Algorithm: the quadratic form depends only on the symmetric part of V
in (d,e), so the D*E contraction folds to D(D+1)/2 unordered pairs
enumerated by circulant offset o: pair (d, e=(d+o)%512) for o in
[0,256].  Host pre-folds coefficients Sh[(o,d),u] = V[d,u,e]+V[e,u,d]
(o=0 diag: V[d,u,d]).  This HALVES the PE FLOPs vs the naive einsum.

Sharding: by contraction - core c owns offsets o in [32c, 32c+32);
every core computes a partial full [U,B] output and the host sums the
8 partials (the unshard step for contraction sharding).  Leftover work
rides one extra single-chunk "mini" unit per core, SPMD-uniform with
per-core data only:
  cores 0-3 : linear term chunk  (minA = x^T rows, minB = ones,
              coeff = W rows)
  core  4   : bias as rank-1     (minA = minB = ones, coeff row0 = b)
  core  5   : idle (zero coeffs)
  cores 6,7 : the o=256 half-offset pairs
Per-core, per o-unit:
  DMA : xw = 512-row window of the wrap-extended x^T at offset o
        (contiguous slabs - no on-device shuffles) + coeff block.
  DVE : G = xT .* xw  (pair products, one bf16 tensor_mul)
  PE  : 4 k-chunks x 4 u-blocks x 2 b-halves matmuls (N=512)
        accumulating into 8 PSUM banks across the whole kernel.
Tail: last unit loops accumulator-major so PSUM banks finish
staggered; evac copies alternate Vector/Scalar and stream out per
slice.  129 chunks/core = 1032 matmuls ~ 223 us of pure PE at the
warm roofline (216 ns per 128x128x512 bf16 matmul).
"""

import sys
import types

import numpy as np
import ml_dtypes

B, D, U = 1024, 512, 512
N_CORES = 8
P = 128                  # partitions
KC = D // P              # k-chunks per unit = 4
NJ = 32                  # o-units per core

BF16 = ml_dtypes.bfloat16


def _ensure_axon_hooks():
    """Provide the antenv.axon_hooks registry if the image lacks it."""
    try:
        import antenv.axon_hooks  # noqa: F401
        return
    except ImportError:
        pass
    mod = types.ModuleType("antenv.axon_hooks")
    mod._hook = None

    def set_axon_ntff_profile_hook(h):
        mod._hook = h

    def get_axon_ntff_profile_hook():
        return mod._hook

    mod.set_axon_ntff_profile_hook = set_axon_ntff_profile_hook
    mod.get_axon_ntff_profile_hook = get_axon_ntff_profile_hook
    sys.modules["antenv.axon_hooks"] = mod
    try:
        import antenv
        antenv.axon_hooks = mod
    except ImportError:
        pass
    try:
        from trn_agent_boot.trn_boot import _ntff_profile_via_ctypes
        hook = _ntff_profile_via_ctypes("/opt/axon/libaxon_pjrt.so")
        if hook is not None:
            set_axon_ntff_profile_hook(hook)
    except Exception:
        pass


def _split_multi_waits(nc, mybir, max_waits=1):
    """Legalize for walrus builds that allow only one sync wait per
    instruction: move extra waits onto same-engine NoOps placed just
    before the offending instruction (queues are in-order, so this is
    semantics-preserving)."""
    for f in nc.m.functions:
        for blk in f.blocks:
            new_insts, changed = [], False
            for inst in blk.instructions:
                si = inst.sync_info
                if si is not None and len(si.on_wait) > max_waits:
                    waits = list(si.on_wait)
                    extra, keep = waits[:-max_waits], waits[-max_waits:]
                    for j, w in enumerate(extra):
                        new_insts.append(mybir.InstNoOp(
                            name=f"{inst.name}-sw{j}",
                            engine=inst.engine,
                            bass_nofuse=True,
                            sync_info=mybir.SyncInfo(on_wait=[w], on_update=[]),
                        ))
                    inst.sync_info = mybir.SyncInfo(
                        on_wait=keep, on_update=list(si.on_update))
                    changed = True
                new_insts.append(inst)
            if changed:
                blk.instructions = new_insts


def _build_program():
    import concourse.bass as bass
    import concourse.mybir as mybir
    import concourse.tile as tile

    f32 = mybir.dt.float32
    bf16 = mybir.dt.bfloat16
    Copy = mybir.ActivationFunctionType.Copy

    nc = bass.Bass(trn_type="TRN2")
    xTc = nc.dram_tensor("xTc", [P, KC, B], bf16, kind="ExternalInput")
    xE = nc.dram_tensor("xE", [544, B], bf16, kind="ExternalInput")
    Sh = nc.dram_tensor("Sh", [NJ, P, KC, U], bf16, kind="ExternalInput")
    gMini = nc.dram_tensor("gMini", [P, B], bf16, kind="ExternalInput")
    g01 = nc.dram_tensor("g01", [2, P, KC, B], bf16, kind="ExternalInput")
    shMini = nc.dram_tensor("shMini", [P, U], bf16, kind="ExternalInput")
    outs = nc.dram_tensor("outs", [U, B], f32, kind="ExternalOutput")

    with tile.TileContext(nc) as tc:
        with tc.tile_pool(name="const", bufs=1) as cpool:
            xT_sb = cpool.tile([P, KC, B], bf16)
            out_sb = cpool.tile([P, KC, B], f32)
            ms_sb = cpool.tile([P, U], bf16)
            gm_sb = cpool.tile([P, B], bf16)
            zW = cpool.tile([P, P], bf16)
            zR = cpool.tile([P, 512], bf16)

            # mini unit first: two tiny loads so the PE starts almost
            # immediately while the big unit-0 windows stream in.
            nc.sync.dma_start(out=gm_sb, in_=gMini[:, :])
            nc.scalar.dma_start(out=ms_sb, in_=shMini[:, :])
            nc.vector.memset(zW, 0.0)
            nc.vector.memset(zR, 0.0)

            with tc.tile_pool(name="wp", bufs=5) as wpool, \
                 tc.tile_pool(name="gp", bufs=4) as gpool, \
                 tc.tile_pool(name="sp", bufs=6) as spool, \
                 tc.tile_pool(name="ap", bufs=1, space="PSUM") as apool:
                accs = [[None, None] for _ in range(4)]
                for ub in range(4):
                    for h in range(2):
                        acc_t = apool.tile([P, 512], f32, tag=f"acc{ub}_{h}")
                        accs[ub][h] = acc_t

                # HAM warm-up: zero-weight matmuls gated only on the
                # memsets keep the PE busy (and un-throttled: the HAM
                # clock gate opens after ~3.4us of activity) while the
                # first real operands stream in.  Zero stationary
                # means they accumulate exact zeros into the banks;
                # one per bank also provides each bank's start=True.
                for i in range(8):
                    nc.tensor.matmul(
                        accs[i % 4][i // 4], zW, zR,
                        start=True, stop=False)
                for ub in range(4):
                    for h in range(2):
                        nc.tensor.matmul(
                            accs[ub][h],
                            ms_sb[:, ub * P:(ub + 1) * P],
                            gm_sb[:, h * 512:(h + 1) * 512],
                            start=False, stop=False)

                # Window chunks k=0..2 ride the sync HWDGE ring,
                # k=3 + coefficients the scalar ring, x^T the gpsimd
                # ring (~25MB per HWDGE ring).  The first two units
                # stay per-k-granular so the PE ramps without waiting
                # for whole tiles; later units use batched DMAs to
                # keep the sequencers' descriptor-gen load low.
                for j in range(NJ):
                    g = gpool.tile([P, KC, B], bf16, tag="g")
                    sh = spool.tile([P, KC, U], bf16, tag="sh")
                    if j < 2:
                        # first two units: host-staged pair products
                        # (same bytes as their windows) - no TT, no
                        # xT dependency on the critical ramp.
                        for k in range(KC):
                            nc.sync.dma_start(
                                out=g[:, k, :], in_=g01[j, :, k, :])
                            nc.scalar.dma_start(
                                out=sh[:, k, :], in_=Sh[j, :, k, :])
                    else:
                        if j == 2:
                            for k in range(KC):
                                nc.scalar.dma_start(
                                    out=xT_sb[:, k, :], in_=xTc[:, k, :])
                        xw = wpool.tile([P, KC, B], bf16, tag="xw")
                        nc.scalar.dma_start(out=sh, in_=Sh[j])
                        for k in range(KC):
                            eng = nc.sync if k < 3 else nc.scalar
                            eng.dma_start(
                                out=xw[:, k, :],
                                in_=xE[j + P * k: j + P * (k + 1), :])
                            nc.vector.tensor_mul(
                                g[:, k, :], xT_sb[:, k, :], xw[:, k, :])
                    if j < NJ - 1:
                        for k in range(KC):
                            for ub in range(4):
                                for h in range(2):
                                    nc.tensor.matmul(
                                        accs[ub][h],
                                        sh[:, k, ub * P:(ub + 1) * P],
                                        g[:, k, h * 512:(h + 1) * 512],
                                        start=False, stop=False)
                    else:
                        # last unit: accumulator-major so PSUM banks
                        # retire staggered and evac overlaps the tail.
                        for ub in range(4):
                            for h in range(2):
                                for k in range(KC):
                                    nc.tensor.matmul(
                                        accs[ub][h],
                                        sh[:, k, ub * P:(ub + 1) * P],
                                        g[:, k, h * 512:(h + 1) * 512],
                                        start=False, stop=(k == KC - 1))

                outs_r = outs.rearrange("(ub p) b -> p ub b", p=P)
                i = 0
                for ub in range(4):
                    for h in range(2):
                        dst = out_sb[:, ub, h * 512:(h + 1) * 512]
                        if i % 2 == 0:
                            nc.vector.tensor_copy(dst, accs[ub][h])
                        else:
                            nc.scalar.activation(dst, accs[ub][h], Copy)
                        nc.sync.dma_start(
                            out=outs_r[:, ub, h * 512:(h + 1) * 512],
                            in_=dst)
                        i += 1

    _split_multi_waits(nc, mybir, max_waits=1)
    return nc


def _host_inputs(x, W, V, b):
    """Build the per-core input arrays (all host-side prep)."""
    xT_bf = np.ascontiguousarray(x.T).astype(BF16)          # [D, B]
    xT_ext = np.concatenate([xT_bf, xT_bf[:256]], axis=0)   # [768, B]
    xTc_np = np.ascontiguousarray(
        xT_bf.reshape(KC, P, B).transpose(1, 0, 2))         # [P, KC, B]

    # folded symmetric coefficients
    Vt = V.transpose(0, 2, 1)                               # [d, e, u]
    Ssum = Vt + Vt.transpose(1, 0, 2)                       # V[d,u,e]+V[e,u,d]
    dd = np.arange(D)
    Vdiag = V[dd, :, dd]                                    # [d, u]

    ones = np.ones((P, B), dtype=BF16)
    zeros = np.zeros((P, B), dtype=BF16)

    def unit_block(M):      # [d, u] -> [p, k, u]
        return M.reshape(KC, P, U).transpose(1, 0, 2)

    in_maps = []
    for c in range(N_CORES):
        Sh_np = np.zeros((NJ, P, KC, U), dtype=np.float32)
        for j in range(NJ):
            o = 32 * c + j
            M = Vdiag if o == 0 else Ssum[dd, (dd + o) % D, :]
            Sh_np[j] = unit_block(M)

        mini_s = np.zeros((P, U), dtype=np.float32)
        if c < 4:                       # linear term, chunk c
            gm = xT_bf[P * c: P * (c + 1)]
            mini_s = W[P * c: P * (c + 1), :].astype(np.float32)
        elif c == 4:                    # bias as rank-1 with ones
            gm = ones
            mini_s[0, :] = b
        elif c == 5:                    # idle
            gm = zeros
        else:                           # o=256 pairs, halves on 6 and 7
            d0 = P * (c - 6)
            gm = (xT_bf[d0: d0 + P] * xT_bf[d0 + 256: d0 + 256 + P]
                  ).astype(BF16)
            mini_s = Ssum[dd[d0:d0 + P], dd[d0:d0 + P] + 256, :]

        xE_c = np.ascontiguousarray(xT_ext[32 * c: 32 * c + 544])
        g01_c = np.empty((2, P, KC, B), dtype=BF16)
        for j in range(2):
            for k in range(KC):
                g01_c[j, :, k, :] = (
                    xTc_np[:, k, :] * xE_c[j + P * k: j + P * (k + 1)])

        in_maps.append({
            "xTc": xTc_np,
            "xE": xE_c,
            "Sh": Sh_np.astype(BF16),
            "gMini": np.ascontiguousarray(gm),
            "g01": g01_c,
            "shMini": mini_s.astype(BF16),
        })
    return in_maps


_LAST_RUN = {}


def kernel(x, W, V, b):
    _ensure_axon_hooks()
    import concourse.bass_utils as bass_utils
    bass_utils.upload_artifacts = lambda d: f"local:{d}"

    x = np.asarray(x, dtype=np.float32)
    W = np.asarray(W, dtype=np.float32)
    V = np.asarray(V, dtype=np.float32)
    b = np.asarray(b, dtype=np.float32)

    in_maps = _host_inputs(x, W, V, b)

    nc = _build_program()
    res = None
    last_exc = None
    for attempt in range(3):
        try:
            res = bass_utils.run_bass_kernel_spmd(
                nc, in_maps, core_ids=list(range(N_CORES)))
            break
        except Exception as e:  # transient NRT device errors have been seen
            last_exc = e
    if res is None:
        raise last_exc
    _LAST_RUN["result"] = res

    acc = np.zeros((U, B), dtype=np.float64)
    for c in range(N_CORES):
        acc += res.results[c]["outs"]
    return np.ascontiguousarray(acc.T).astype(np.float32)


# revision 29
# speedup vs baseline: 1.0522x; 1.0522x over previous
"""Trainium2 Bass kernel for nn_DenseTensor (dense_mlp, bilinear form).

Computes out = x @ W + einsum('bd,due,be->bu', x, V, x) + b with
B=1024, D=U=E=512 on 8 NeuronCores.

Algorithm: the quadratic form depends only on the symmetric part of V
in (d,e), so the D*E contraction folds to D(D+1)/2 unordered pairs
enumerated by circulant offset o: pair (d, e=(d+o)%512) for o in
[0,256].  Host pre-folds coefficients Sh[(o,d),u] = V[d,u,e]+V[e,u,d]
(o=0 diag: V[d,u,d]).  This HALVES the PE FLOPs vs the naive einsum.

Sharding: by contraction - core c owns offsets o in [32c, 32c+32);
every core computes a partial full [U,B] output and the host sums the
8 partials (the unshard step for contraction sharding).  Leftover work
rides one extra single-chunk "mini" unit per core, SPMD-uniform with
per-core data only:
  cores 0-3 : linear term chunk  (minA = x^T rows, minB = ones,
              coeff = W rows)
  core  4   : bias as rank-1     (minA = minB = ones, coeff row0 = b)
  core  5   : idle (zero coeffs)
  cores 6,7 : the o=256 half-offset pairs
Per-core, per o-unit:
  DMA : xw = 512-row window of the wrap-extended x^T at offset o
        (contiguous slabs - no on-device shuffles) + coeff block.
  DVE : G = xT .* xw  (pair products, one bf16 tensor_mul)
  PE  : 4 k-chunks x 4 u-blocks x 2 b-halves matmuls (N=512)
        accumulating into 8 PSUM banks across the whole kernel.
Tail: last unit loops accumulator-major so PSUM banks finish
staggered; evac copies alternate Vector/Scalar and stream out per
slice.  129 chunks/core = 1032 matmuls ~ 223 us of pure PE at the
warm roofline (216 ns per 128x128x512 bf16 matmul).
"""

import sys
import types

import numpy as np
import ml_dtypes

B, D, U = 1024, 512, 512
N_CORES = 8
P = 128                  # partitions
KC = D // P              # k-chunks per unit = 4
NJ = 32                  # o-units per core

BF16 = ml_dtypes.bfloat16


def _ensure_axon_hooks():
    """Provide the antenv.axon_hooks registry if the image lacks it."""
    try:
        import antenv.axon_hooks  # noqa: F401
        return
    except ImportError:
        pass
    mod = types.ModuleType("antenv.axon_hooks")
    mod._hook = None

    def set_axon_ntff_profile_hook(h):
        mod._hook = h

    def get_axon_ntff_profile_hook():
        return mod._hook

    mod.set_axon_ntff_profile_hook = set_axon_ntff_profile_hook
    mod.get_axon_ntff_profile_hook = get_axon_ntff_profile_hook
    sys.modules["antenv.axon_hooks"] = mod
    try:
        import antenv
        antenv.axon_hooks = mod
    except ImportError:
        pass
    try:
        from trn_agent_boot.trn_boot import _ntff_profile_via_ctypes
        hook = _ntff_profile_via_ctypes("/opt/axon/libaxon_pjrt.so")
        if hook is not None:
            set_axon_ntff_profile_hook(hook)
    except Exception:
        pass


def _split_multi_waits(nc, mybir, max_waits=1):
    """Legalize for walrus builds that allow only one sync wait per
    instruction: move extra waits onto same-engine NoOps placed just
    before the offending instruction (queues are in-order, so this is
    semantics-preserving)."""
    for f in nc.m.functions:
        for blk in f.blocks:
            new_insts, changed = [], False
            for inst in blk.instructions:
                si = inst.sync_info
                if si is not None and len(si.on_wait) > max_waits:
                    waits = list(si.on_wait)
                    extra, keep = waits[:-max_waits], waits[-max_waits:]
                    for j, w in enumerate(extra):
                        new_insts.append(mybir.InstNoOp(
                            name=f"{inst.name}-sw{j}",
                            engine=inst.engine,
                            bass_nofuse=True,
                            sync_info=mybir.SyncInfo(on_wait=[w], on_update=[]),
                        ))
                    inst.sync_info = mybir.SyncInfo(
                        on_wait=keep, on_update=list(si.on_update))
                    changed = True
                new_insts.append(inst)
            if changed:
                blk.instructions = new_insts


def _build_program():
    import concourse.bass as bass
    import concourse.mybir as mybir
    import concourse.tile as tile

    f32 = mybir.dt.float32
    bf16 = mybir.dt.bfloat16
    Copy = mybir.ActivationFunctionType.Copy

    nc = bass.Bass(trn_type="TRN2")
    xTc = nc.dram_tensor("xTc", [P, KC, B], bf16, kind="ExternalInput")
    xE = nc.dram_tensor("xE", [544, B], bf16, kind="ExternalInput")
    Sh = nc.dram_tensor("Sh", [NJ, P, KC, U], bf16, kind="ExternalInput")
    gMini = nc.dram_tensor("gMini", [P, B], bf16, kind="ExternalInput")
    g01 = nc.dram_tensor("g01", [2, P, KC, B], bf16, kind="ExternalInput")
    shMini = nc.dram_tensor("shMini", [P, U], bf16, kind="ExternalInput")
    outs = nc.dram_tensor("outs", [U, B], f32, kind="ExternalOutput")

    with tile.TileContext(nc) as tc:
        with tc.tile_pool(name="const", bufs=1) as cpool:
            xT_sb = cpool.tile([P, KC, B], bf16)
            out_sb = cpool.tile([P, KC, B], f32)
            ms_sb = cpool.tile([P, U], bf16)
            gm_sb = cpool.tile([P, B], bf16)
            zW = cpool.tile([P, P], bf16)
            zR = cpool.tile([P, 512], bf16)

            # mini unit first: two tiny loads so the PE starts almost
            # immediately while the big unit-0 windows stream in.
            nc.sync.dma_start(out=gm_sb, in_=gMini[:, :])
            nc.scalar.dma_start(out=ms_sb, in_=shMini[:, :])
            nc.vector.memset(zW, 0.0)
            nc.vector.memset(zR, 0.0)

            with tc.tile_pool(name="wp", bufs=6) as wpool, \
                 tc.tile_pool(name="gp", bufs=5) as gpool, \
                 tc.tile_pool(name="sp", bufs=8) as spool, \
                 tc.tile_pool(name="ap", bufs=1, space="PSUM") as apool:
                accs = [[None, None] for _ in range(4)]
                for ub in range(4):
                    for h in range(2):
                        acc_t = apool.tile([P, 512], f32, tag=f"acc{ub}_{h}")
                        accs[ub][h] = acc_t

                # HAM warm-up: zero-weight matmuls gated only on the
                # memsets keep the PE busy (and un-throttled: the HAM
                # clock gate opens after ~3.4us of activity) while the
                # first real operands stream in.  Zero stationary
                # means they accumulate exact zeros into the banks;
                # one per bank also provides each bank's start=True.
                for i in range(8):
                    nc.tensor.matmul(
                        accs[i % 4][i // 4], zW, zR,
                        start=True, stop=False)
                for ub in range(4):
                    for h in range(2):
                        nc.tensor.matmul(
                            accs[ub][h],
                            ms_sb[:, ub * P:(ub + 1) * P],
                            gm_sb[:, h * 512:(h + 1) * 512],
                            start=False, stop=False)

                # Window chunks k=0..2 ride the sync HWDGE ring,
                # k=3 + coefficients the scalar ring, x^T the gpsimd
                # ring (~25MB per HWDGE ring).  The first two units
                # stay per-k-granular so the PE ramps without waiting
                # for whole tiles; later units use batched DMAs to
                # keep the sequencers' descriptor-gen load low.
                for j in range(NJ):
                    g = gpool.tile([P, KC, B], bf16, tag="g")
                    sh = spool.tile([P, KC, U], bf16, tag="sh")
                    if j < 2:
                        # first two units: host-staged pair products
                        # (same bytes as their windows) - no TT, no
                        # xT dependency on the critical ramp.
                        for k in range(KC):
                            nc.sync.dma_start(
                                out=g[:, k, :], in_=g01[j, :, k, :])
                            nc.scalar.dma_start(
                                out=sh[:, k, :], in_=Sh[j, :, k, :])
                    else:
                        if j == 2:
                            for k in range(KC):
                                nc.scalar.dma_start(
                                    out=xT_sb[:, k, :], in_=xTc[:, k, :])
                        xw = wpool.tile([P, KC, B], bf16, tag="xw")
                        nc.scalar.dma_start(out=sh, in_=Sh[j])
                        for k in range(KC):
                            eng = nc.sync if k < 3 else nc.scalar
                            eng.dma_start(
                                out=xw[:, k, :],
                                in_=xE[j + P * k: j + P * (k + 1), :])
                            nc.vector.tensor_mul(
                                g[:, k, :], xT_sb[:, k, :], xw[:, k, :])
                    if j < NJ - 1:
                        for k in range(KC):
                            for ub in range(4):
                                for h in range(2):
                                    nc.tensor.matmul(
                                        accs[ub][h],
                                        sh[:, k, ub * P:(ub + 1) * P],
                                        g[:, k, h * 512:(h + 1) * 512],
                                        start=False, stop=False)
                    else:
                        # last unit: accumulator-major so PSUM banks
                        # retire staggered and evac overlaps the tail.
                        for ub in range(4):
                            for h in range(2):
                                for k in range(KC):
                                    nc.tensor.matmul(
                                        accs[ub][h],
                                        sh[:, k, ub * P:(ub + 1) * P],
                                        g[:, k, h * 512:(h + 1) * 512],
                                        start=False, stop=(k == KC - 1))

                outs_r = outs.rearrange("(ub p) b -> p ub b", p=P)
                i = 0
                for ub in range(4):
                    for h in range(2):
                        dst = out_sb[:, ub, h * 512:(h + 1) * 512]
                        if i % 2 == 0:
                            nc.vector.tensor_copy(dst, accs[ub][h])
                        else:
                            nc.scalar.activation(dst, accs[ub][h], Copy)
                        nc.sync.dma_start(
                            out=outs_r[:, ub, h * 512:(h + 1) * 512],
                            in_=dst)
                        i += 1

    _split_multi_waits(nc, mybir, max_waits=1)
    return nc


def _host_inputs(x, W, V, b):
    """Build the per-core input arrays (all host-side prep)."""
    xT_bf = np.ascontiguousarray(x.T).astype(BF16)          # [D, B]
    xT_ext = np.concatenate([xT_bf, xT_bf[:256]], axis=0)   # [768, B]
    xTc_np = np.ascontiguousarray(
        xT_bf.reshape(KC, P, B).transpose(1, 0, 2))         # [P, KC, B]

    # folded symmetric coefficients
    Vt = V.transpose(0, 2, 1)                               # [d, e, u]
    Ssum = Vt + Vt.transpose(1, 0, 2)                       # V[d,u,e]+V[e,u,d]
    dd = np.arange(D)
    Vdiag = V[dd, :, dd]                                    # [d, u]

    ones = np.ones((P, B), dtype=BF16)
    zeros = np.zeros((P, B), dtype=BF16)

    def unit_block(M):      # [d, u] -> [p, k, u]
        return M.reshape(KC, P, U).transpose(1, 0, 2)

    in_maps = []
    for c in range(N_CORES):
        Sh_np = np.zeros((NJ, P, KC, U), dtype=np.float32)
        for j in range(NJ):
            o = 32 * c + j
            M = Vdiag if o == 0 else Ssum[dd, (dd + o) % D, :]
            Sh_np[j] = unit_block(M)

        mini_s = np.zeros((P, U), dtype=np.float32)
        if c < 4:                       # linear term, chunk c
            gm = xT_bf[P * c: P * (c + 1)]
            mini_s = W[P * c: P * (c + 1), :].astype(np.float32)
        elif c == 4:                    # bias as rank-1 with ones
            gm = ones
            mini_s[0, :] = b
        elif c == 5:                    # idle
            gm = zeros
        else:                           # o=256 pairs, halves on 6 and 7
            d0 = P * (c - 6)
            gm = (xT_bf[d0: d0 + P] * xT_bf[d0 + 256: d0 + 256 + P]
                  ).astype(BF16)
            mini_s = Ssum[dd[d0:d0 + P], dd[d0:d0 + P] + 256, :]

        xE_c = np.ascontiguousarray(xT_ext[32 * c: 32 * c + 544])
        g01_c = np.empty((2, P, KC, B), dtype=BF16)
        for j in range(2):
            for k in range(KC):
                g01_c[j, :, k, :] = (
                    xTc_np[:, k, :] * xE_c[j + P * k: j + P * (k + 1)])

        in_maps.append({
            "xTc": xTc_np,
            "xE": xE_c,
            "Sh": Sh_np.astype(BF16),
            "gMini": np.ascontiguousarray(gm),
            "g01": g01_c,
            "shMini": mini_s.astype(BF16),
        })
    return in_maps


_LAST_RUN = {}


def kernel(x, W, V, b):
    _ensure_axon_hooks()
    import concourse.bass_utils as bass_utils
    bass_utils.upload_artifacts = lambda d: f"local:{d}"

    x = np.asarray(x, dtype=np.float32)
    W = np.asarray(W, dtype=np.float32)
    V = np.asarray(V, dtype=np.float32)
    b = np.asarray(b, dtype=np.float32)

    in_maps = _host_inputs(x, W, V, b)

    nc = _build_program()
    res = None
    last_exc = None
    for attempt in range(3):
        try:
            res = bass_utils.run_bass_kernel_spmd(
                nc, in_maps, core_ids=list(range(N_CORES)))
            break
        except Exception as e:  # transient NRT device errors have been seen
            last_exc = e
    if res is None:
        raise last_exc
    _LAST_RUN["result"] = res

    acc = np.zeros((U, B), dtype=np.float64)
    for c in range(N_CORES):
        acc += res.results[c]["outs"]
    return np.ascontiguousarray(acc.T).astype(np.float32)
